# revision 5
# baseline (speedup 1.0000x reference)
"""Causal self-attention (B=4, S=2048, D=1024, single head, fp32) on 8 trn2
NeuronCores.

Sharding: core 2*b + c handles batch b with the parity-c half of the keys
(global key rows 2*i + c), over ALL queries — a flash-attention split over
the key dimension. Each core returns unnormalized softmax numerators
o = sum_k exp(s~ - m~) v plus per-row stats (m = raw-score row max,
l = sum exp); the host combines the two key-halves exactly.

SPMD trick: one program serves both parities. The host pair-swaps the rows
of x for odd cores (rows [1,0,3,2,...]), so each core's keys sit at even
row positions and the on-chip stride-2 access pattern is parity-free. The
causal boundary masks (which depend on the parity) ship as a small
per-core input; the host pair-swaps the outputs of odd cores back.

For query block j (128 rows) the valid compacted key blocks are 0..j//2,
only the last one partially masked — identical structure for every j
parity class, so the fully unrolled program is the same on all cores.

Matmuls run in float32r (full PE rate at N=512, ~16x more accurate than
bf16); attn @ v runs in bf16 (attn in [0,1], v ~ N(0,1)).
"""
import math
import numpy as np

import concourse.bacc as bacc
import concourse.mybir as mybir
from concourse import tile
from concourse.masks import make_identity
from concourse.bass_utils import run_bass_kernel_spmd

B, S, D = 4, 2048, 1024
P = 128
DT = D // P          # 8 d-tiles (contraction)
ET = D // P          # 8 e-tiles (output feature)
ST = S // P          # 16 s-tiles (sequence)
HKT = ST // 2        # 8 compacted key tiles per core
NQB = S // P         # 16 query blocks
INV_SQRT_D = 1.0 / math.sqrt(D)
NEG = -1e30

F32 = mybir.dt.float32
F32R = mybir.dt.float32r
BF16 = mybir.dt.bfloat16

_CACHED_NC = None


def _ceil_div(a, b):
    return (a + b - 1) // b


def build_nc():
    nc = bacc.Bacc("TRN2", target_bir_lowering=False)
    x_p = nc.declare_dram_parameter("x", [S, D], F32, isOutput=False)
    wq_p = nc.declare_dram_parameter("wq", [D, D], F32, isOutput=False)
    wk_p = nc.declare_dram_parameter("wk", [D, D], F32, isOutput=False)
    wv_p = nc.declare_dram_parameter("wv", [D, D], F32, isOutput=False)
    mask_p = nc.declare_dram_parameter("mask", [P, 2, P], F32, isOutput=False)
    o_p = nc.declare_dram_parameter("o", [S, D], F32, isOutput=True)
    m_p = nc.declare_dram_parameter("m", [S, 1], F32, isOutput=True)
    l_p = nc.declare_dram_parameter("l", [S, 1], F32, isOutput=True)

    with tile.TileContext(nc) as tc:
        # ---- persistent pools (bottom of SBUF stack) ----
        with (
            tc.tile_pool(name="qT_pool", bufs=1) as qT_pool,
            tc.tile_pool(name="kT_pool", bufs=1) as kT_pool,
            tc.tile_pool(name="v_pool", bufs=1) as v_pool,
            tc.tile_pool(name="const_pool", bufs=1) as const_pool,
        ):
            qT = qT_pool.tile([P, ET, S], F32R)        # [e_p, et, s_q] 64KB/p
            kT = kT_pool.tile([P, ET, HKT * P], F32R)  # [e_p, et, s_k] 32KB/p
            vv = v_pool.tile([P, HKT, D], BF16)        # [s_k_p, st, e] 16KB/p
            ident_f32 = const_pool.tile([P, P], F32)
            ident_bf = const_pool.tile([P, P], BF16)
            mask_sb = const_pool.tile([P, 2, P], F32)
            make_identity(nc, ident_f32[:])
            make_identity(nc, ident_bf[:])
            nc.sync.dma_start(out=mask_sb[:], in_=mask_p[:])

            # ================= Phase A: x^T + projections =================
            with tc.tile_pool(name="xT_pool", bufs=1) as xT_pool:
                xT = xT_pool.tile([P, DT, S], F32R)    # [d_p, dt, s] 64KB/p

                # A1: transpose x into xT (PE transpose per 128x128 tile)
                with (
                    tc.tile_pool(name="xs_pool", bufs=3) as xs_pool,
                    tc.tile_pool(name="psT_pool", bufs=4, space="PSUM") as psT_pool,
                ):
                    for st in range(ST):
                        x_f32 = xs_pool.tile([P, D], F32, tag="xs",
                                             name=f"xs{st}")
                        nc.sync.dma_start(
                            out=x_f32[:], in_=x_p[st * P:(st + 1) * P, :])
                        for dt in range(DT):
                            psT = psT_pool.tile([P, P], F32, tag="psT",
                                                name=f"psT{st}_{dt}")
                            nc.tensor.transpose(
                                psT[:], x_f32[:, dt * P:(dt + 1) * P],
                                ident_f32[:])
                            nc.vector.tensor_copy(
                                xT[:, dt, st * P:(st + 1) * P], psT[:])

                # even-position (this core's keys) stride-2 view of xT
                xT_keys = xT.rearrange("p d (s two) -> p d two s", two=2)

                # A2: kT[e, i] = sum_d Wk[d, e] * x_key[i, d]
                with (
                    tc.tile_pool(name="wkq_pool", bufs=2) as wkq_pool,
                    tc.tile_pool(name="psK_pool", bufs=2, space="PSUM") as psK_pool,
                ):
                    for et in range(ET):
                        wk_f = wkq_pool.tile([P, DT, P], F32, tag="wf",
                                             name=f"wkf{et}")
                        wk_r = wkq_pool.tile([P, DT, P], F32R, tag="wr",
                                             name=f"wkr{et}")
                        nc.sync.dma_start(
                            out=wk_f[:],
                            in_=wk_p[:, et * P:(et + 1) * P].rearrange(
                                "(dt p) e -> p dt e", p=P))
                        nc.vector.tensor_copy(wk_r[:], wk_f[:])
                        pss = [psK_pool.tile([P, 512], F32, tag=f"psK{ch}",
                                             name=f"psK{et}_{ch}")
                               for ch in range(2)]
                        for d in range(DT):
                            for ch in range(2):
                                nc.tensor.matmul(
                                    pss[ch][:],
                                    wk_r[:, d, :],
                                    xT_keys[:, d, 0, ch * 512:(ch + 1) * 512],
                                    start=(d == 0), stop=(d == DT - 1))
                        for ch in range(2):
                            nc.vector.tensor_copy(
                                kT[:, et, ch * 512:(ch + 1) * 512], pss[ch][:])

                # A3: v[i, e] = sum_d x_key[i, d] * Wv[d, e]   (8 psum banks)
                with (
                    tc.tile_pool(name="wv_pool", bufs=2) as wv_pool,
                    tc.tile_pool(name="psV_pool", bufs=1, space="PSUM") as psV_pool,
                ):
                    for eb in range(2):
                        pss = [psV_pool.tile([P, 512], F32, tag=f"psV{st}",
                                             name=f"psV{eb}_{st}")
                               for st in range(HKT)]
                        for d in range(DT):
                            wv_f = wv_pool.tile([P, 512], F32, tag="wvf",
                                                name=f"wvf{eb}_{d}")
                            wv_r = wv_pool.tile([P, 512], F32R, tag="wvr",
                                                name=f"wvr{eb}_{d}")
                            nc.sync.dma_start(
                                out=wv_f[:],
                                in_=wv_p[d * P:(d + 1) * P,
                                         eb * 512:(eb + 1) * 512])
                            nc.vector.tensor_copy(wv_r[:], wv_f[:])
                            for st in range(HKT):
                                nc.tensor.matmul(
                                    pss[st][:],
                                    xT_keys[:, d, 0, st * P:(st + 1) * P],
                                    wv_r[:],
                                    start=(d == 0), stop=(d == DT - 1))
                        for st in range(HKT):
                            nc.vector.tensor_copy(
                                vv[:, st, eb * 512:(eb + 1) * 512], pss[st][:])

                # A4: qT[e, s] = sum_d Wq[d, e] * x[s, d]  (all queries)
                with (
                    tc.tile_pool(name="wq_pool", bufs=2) as wq_pool,
                    tc.tile_pool(name="psQ_pool", bufs=2, space="PSUM") as psQ_pool,
                ):
                    for et in range(ET):
                        wq_f = wq_pool.tile([P, DT, P], F32, tag="wf",
                                            name=f"wqf{et}")
                        wq_r = wq_pool.tile([P, DT, P], F32R, tag="wr",
                                            name=f"wqr{et}")
                        nc.sync.dma_start(
                            out=wq_f[:],
                            in_=wq_p[:, et * P:(et + 1) * P].rearrange(
                                "(dt p) e -> p dt e", p=P))
                        nc.vector.tensor_copy(wq_r[:], wq_f[:])
                        pss = [psQ_pool.tile([P, 512], F32, tag=f"psQ{ch}",
                                             name=f"psQ{et}_{ch}")
                               for ch in range(4)]
                        for d in range(DT):
                            for ch in range(4):
                                nc.tensor.matmul(
                                    pss[ch][:],
                                    wq_r[:, d, :],
                                    xT[:, d, ch * 512:(ch + 1) * 512],
                                    start=(d == 0), stop=(d == DT - 1))
                        for ch in range(4):
                            nc.vector.tensor_copy(
                                qT[:, et, ch * 512:(ch + 1) * 512], pss[ch][:])

            # ================= Phase B: causal attention =================
            with (
                tc.tile_pool(name="sc_pool", bufs=2) as sc_pool,
                tc.tile_pool(name="at_pool", bufs=2) as at_pool,
                tc.tile_pool(name="atT_pool", bufs=4) as atT_pool,
                tc.tile_pool(name="st_pool", bufs=4) as st_pool,
                tc.tile_pool(name="ob_pool", bufs=2) as ob_pool,
                tc.tile_pool(name="psS_pool", bufs=1, space="PSUM") as psS_pool,
                tc.tile_pool(name="psA_pool", bufs=3, space="PSUM") as psA_pool,
                tc.tile_pool(name="psO_pool", bufs=1, space="PSUM") as psO_pool,
            ):
                for j in range(NQB):
                    nkb = j // 2 + 1          # valid compacted key blocks
                    ncols = nkb * P
                    nch = _ceil_div(ncols, 512)
                    scores = sc_pool.tile([P, HKT * P], F32, tag="scores",
                                          name=f"scores{j}")
                    attn = at_pool.tile([P, HKT * P], BF16, tag="attn",
                                        name=f"attn{j}")

                    # scores = qT[:, j-block]^T @ kT  (contract over e)
                    for ch in range(nch):
                        ncc = min(512, ncols - ch * 512)
                        psS = psS_pool.tile([P, 512], F32, tag=f"psS{ch % 2}",
                                            name=f"psS{j}_{ch}")
                        for et in range(ET):
                            nc.tensor.matmul(
                                psS[:, :ncc],
                                qT[:, et, j * P:(j + 1) * P],
                                kT[:, et, ch * 512:ch * 512 + ncc],
                                start=(et == 0), stop=(et == ET - 1))
                        # evict to scores; boundary block gets the causal mask
                        lo, hi = ch * 512, ch * 512 + ncc
                        if hi == ncols:
                            if ncc > P:
                                nc.scalar.copy(scores[:, lo:hi - P],
                                               psS[:, :ncc - P])
                            nc.vector.tensor_add(
                                scores[:, hi - P:hi],
                                psS[:, ncc - P:ncc],
                                mask_sb[:, j % 2, :])
                        else:
                            nc.scalar.copy(scores[:, lo:hi], psS[:, :ncc])

                    # softmax over the valid region
                    m_t = st_pool.tile([P, 1], F32, tag="m", name=f"m{j}")
                    neg_t = st_pool.tile([P, 1], F32, tag="neg", name=f"neg{j}")
                    l_t = st_pool.tile([P, 1], F32, tag="l", name=f"l{j}")
                    nc.vector.reduce_max(m_t[:], scores[:, :ncols],
                                         axis=mybir.AxisListType.X)
                    nc.vector.tensor_scalar_mul(neg_t[:], m_t[:], -INV_SQRT_D)
                    nc.scalar.activation(
                        attn[:, :ncols], scores[:, :ncols],
                        mybir.ActivationFunctionType.Exp,
                        bias=neg_t[:], scale=INV_SQRT_D, accum_out=l_t[:])
                    nc.sync.dma_start(out=m_p[j * P:(j + 1) * P, :], in_=m_t[:])
                    nc.sync.dma_start(out=l_p[j * P:(j + 1) * P, :], in_=l_t[:])

                    # o = attn @ v   (transpose attn blocks, contract over keys)
                    atTs = []
                    for kb in range(nkb):
                        psA = psA_pool.tile([P, P], BF16, tag="psA",
                                            name=f"psA{j}_{kb}")
                        atT = atT_pool.tile([P, P], BF16, tag="atT",
                                            name=f"atT{j}_{kb}")
                        nc.tensor.transpose(
                            psA[:], attn[:, kb * P:(kb + 1) * P], ident_bf[:])
                        nc.vector.tensor_copy(atT[:], psA[:])
                        atTs.append(atT)
                    psO = [psO_pool.tile([P, 512], F32, tag=f"psO{eb}",
                                         name=f"psO{j}_{eb}")
                           for eb in range(2)]
                    for kb in range(nkb):
                        for eb in range(2):
                            nc.tensor.matmul(
                                psO[eb][:],
                                atTs[kb][:],
                                vv[:, kb, eb * 512:(eb + 1) * 512],
                                start=(kb == 0), stop=(kb == nkb - 1))
                    for eb in range(2):
                        o_sb = ob_pool.tile([P, 512], F32, tag="o",
                                            name=f"o{j}_{eb}")
                        nc.scalar.copy(o_sb[:], psO[eb][:])
                        nc.sync.dma_start(
                            out=o_p[j * P:(j + 1) * P,
                                    eb * 512:(eb + 1) * 512],
                            in_=o_sb[:])
    nc.finalize()
    return nc


def _boundary_masks(c):
    """mask[row, par, i]: 0 if compacted key i is causally valid for local
    query row `row` of an even (par=0) / odd (par=1) query block, else -1e30.

    For parity-1 cores, x rows arrive pair-swapped, so the query at local
    position `row` is global row 128*j + r_local with
    r_local = row+1 (even row) / row-1 (odd row). Key i is global row
    256*(j//2) + 2*i + c. Valid iff 2*i + c <= par*128 + r_local.
    """
    mask = np.full((P, 2, P), NEG, dtype=np.float32)
    for row in range(P):
        r_local = row if c == 0 else (row + 1 if row % 2 == 0 else row - 1)
        for par in range(2):
            lim = (par * P + r_local - c) // 2
            if lim >= 0:
                mask[row, par, :min(lim + 1, P)] = 0.0
    return mask


_PAIRSWAP = np.arange(S).reshape(-1, 2)[:, ::-1].reshape(-1)


def _make_in_maps(x, Wq, Wk, Wv):
    x = np.asarray(x, dtype=np.float32)
    Wq = np.ascontiguousarray(np.asarray(Wq, dtype=np.float32))
    Wk = np.ascontiguousarray(np.asarray(Wk, dtype=np.float32))
    Wv = np.ascontiguousarray(np.asarray(Wv, dtype=np.float32))
    masks = [_boundary_masks(0), _boundary_masks(1)]
    in_maps = []
    for core in range(8):
        b, c = core // 2, core % 2
        xb = x[b] if c == 0 else x[b][_PAIRSWAP]
        in_maps.append({
            "x": np.ascontiguousarray(xb),
            "wq": Wq, "wk": Wk, "wv": Wv,
            "mask": masks[c],
        })
    return in_maps


def _combine(res):
    out = np.empty((B, S, D), dtype=np.float32)
    for b in range(B):
        r0, r1 = res.results[2 * b], res.results[2 * b + 1]
        o0, m0, l0 = r0["o"], r0["m"], r0["l"]
        # parity-1 core computed on pair-swapped query rows; swap back
        o1 = r1["o"][_PAIRSWAP]
        m1 = r1["m"][_PAIRSWAP]
        l1 = r1["l"][_PAIRSWAP]
        ms0 = m0.astype(np.float64) * INV_SQRT_D
        ms1 = m1.astype(np.float64) * INV_SQRT_D
        mm = np.maximum(ms0, ms1)
        w0 = np.exp(ms0 - mm)
        w1 = np.exp(ms1 - mm)
        num = w0 * o0.astype(np.float64) + w1 * o1.astype(np.float64)
        den = w0 * l0.astype(np.float64) + w1 * l1.astype(np.float64)
        out[b] = (num / den).astype(np.float32)
    return out


def kernel(x, Wq, Wk, Wv):
    global _CACHED_NC
    if _CACHED_NC is None:
        _CACHED_NC = build_nc()
    in_maps = _make_in_maps(x, Wq, Wk, Wv)
    res = run_bass_kernel_spmd(_CACHED_NC, in_maps, list(range(8)))
    return _combine(res)


# revision 9
# speedup vs baseline: 1.2452x; 1.2452x over previous
"""Causal self-attention (B=4, S=2048, D=1024, single head, fp32) on 8 trn2
NeuronCores.

Sharding: core 2*b + c handles batch b with the parity-c half of the keys
(global key rows 2*i + c), over ALL queries — a flash-attention split over
the key dimension. Each core returns unnormalized softmax numerators
o = sum_k exp(s~ - m~) v plus per-row stats (m = raw-score row max,
l = sum exp); the host combines the two key-halves exactly.

SPMD trick: one program serves both parities. The host pair-swaps the rows
of x for odd cores (rows [1,0,3,2,...]), so each core's keys sit at even
row positions and the on-chip stride-2 access pattern is parity-free. The
causal boundary masks (which depend on the parity) ship as a small
per-core input; the host pair-swaps the outputs of odd cores back.

For query block j (128 rows) the valid compacted key blocks are 0..j//2,
only the last one partially masked — identical structure for every j, so
the fully unrolled program is the same on all cores.

Matmuls run in float32r (full PE rate at N=512, ~16x more accurate than
bf16); attn @ v runs in bf16 (attn in [0,1], v ~ N(0,1)). x is transposed
on-chip without the PE: the x DMA applies a 32x32 block permutation and a
DVE stream-transpose finishes each block.
"""
import math
import numpy as np

import concourse.bacc as bacc
import concourse.mybir as mybir
from concourse import tile
from concourse.masks import make_identity
from concourse.bass_utils import run_bass_kernel_spmd

B, S, D = 4, 2048, 1024
P = 128
DT = D // P          # 8 d-tiles (contraction)
ET = D // P          # 8 e-tiles (output feature)
ST = S // P          # 16 s-tiles (sequence)
HKT = ST // 2        # 8 compacted key tiles per core
NQB = S // P         # 16 query blocks
INV_SQRT_D = 1.0 / math.sqrt(D)
NEG = -1e30

F32 = mybir.dt.float32
F32R = mybir.dt.float32r
BF16 = mybir.dt.bfloat16

USE_STREAM_T = True   # x^T via DMA block-permute + DVE stream transpose

_CACHED_NC = None


def _ceil_div(a, b):
    return (a + b - 1) // b


def build_nc():
    nc = bacc.Bacc("TRN2", target_bir_lowering=False)
    x_p = nc.declare_dram_parameter("x", [S, D], F32, isOutput=False)
    wq_p = nc.declare_dram_parameter("wq", [D, D], F32, isOutput=False)
    wk_p = nc.declare_dram_parameter("wk", [D, D], F32, isOutput=False)
    wv_p = nc.declare_dram_parameter("wv", [D, D], F32, isOutput=False)
    mask_p = nc.declare_dram_parameter("mask", [P, 2, P], F32, isOutput=False)
    o_p = nc.declare_dram_parameter("o", [S, D], F32, isOutput=True)
    m_p = nc.declare_dram_parameter("m", [S, 1], F32, isOutput=True)
    l_p = nc.declare_dram_parameter("l", [S, 1], F32, isOutput=True)

    with tile.TileContext(nc) as tc:
        # ---- persistent pools (bottom of SBUF stack) ----
        with (
            tc.tile_pool(name="qT_pool", bufs=1) as qT_pool,
            tc.tile_pool(name="kT_pool", bufs=1) as kT_pool,
            tc.tile_pool(name="v_pool", bufs=1) as v_pool,
            tc.tile_pool(name="const_pool", bufs=1) as const_pool,
        ):
            qT = qT_pool.tile([P, ET, S], F32R)        # [e_p, et, s_q] 64KB/p
            kT = kT_pool.tile([P, ET, HKT * P], F32R)  # [e_p, et, s_k] 32KB/p
            vv = v_pool.tile([P, HKT, D], BF16)        # [s_k_p, st, e] 16KB/p
            ident_f32 = const_pool.tile([P, P], F32)
            ident_bf = const_pool.tile([P, P], BF16)
            mask_sb = const_pool.tile([P, 2, P], F32)
            make_identity(nc, ident_f32[:])
            make_identity(nc, ident_bf[:])
            nc.sync.dma_start(out=mask_sb[:], in_=mask_p[:])

            # ================= Phase A: x^T + projections =================
            with (
                tc.tile_pool(name="xT_pool", bufs=1) as xT_pool,
                tc.tile_pool(name="stage_pool", bufs=2) as stage_pool,
                tc.tile_pool(name="psA_all", bufs=1, space="PSUM") as psAll,
            ):
                xT = xT_pool.tile([P, DT, S], F32R)    # [d_p, dt, s] 64KB/p
                psb = [psAll.tile([P, 512], F32, tag=f"b{i}", name=f"psb{i}")
                       for i in range(8)]

                # A1: x -> x^T
                if USE_STREAM_T:
                    # DMA applies the 32-block grid permute; stream transpose
                    # finishes each 32x32 block on the DVE. PE stays free.
                    for st in range(ST):
                        y_t = stage_pool.tile([P, DT, P], F32, tag="xs",
                                              name=f"xs{st}", bufs=2)
                        z_t = stage_pool.tile([P, DT, P], F32, tag="zs",
                                              name=f"zs{st}", bufs=1)
                        x_r = x_p[st * P:(st + 1) * P, :].rearrange(
                            "(b w) (dt a u) -> a w dt b u",
                            b=4, w=32, dt=DT, a=4, u=32)
                        for a in range(4):
                            nc.sync.dma_start(
                                out=y_t[32 * a:32 * (a + 1), :, :].rearrange(
                                    "w dt (b u) -> w dt b u", b=4),
                                in_=x_r[a])
                        for dt in range(DT):
                            nc.vector.transpose(z_t[:, dt, :], y_t[:, dt, :])
                        # rounding fp32 -> fp32r (required by the PE verifier)
                        nc.vector.tensor_copy(
                            xT[:, :, st * P:(st + 1) * P], z_t[:])
                else:
                    for st in range(ST):
                        x_f32 = stage_pool.tile([P, D], F32, tag="xs",
                                                name=f"xs{st}", bufs=2)
                        nc.sync.dma_start(
                            out=x_f32[:], in_=x_p[st * P:(st + 1) * P, :])
                        for dt in range(DT):
                            ps = psb[(st * DT + dt) % 8]
                            nc.tensor.transpose(
                                ps[:, :P], x_f32[:, dt * P:(dt + 1) * P],
                                ident_f32[:])
                            nc.vector.tensor_copy(
                                xT[:, dt, st * P:(st + 1) * P], ps[:, :P])

                # even-position (this core's keys) stride-2 view of xT
                xT_keys = xT.rearrange("p d (s two) -> p d two s", two=2)

                # A2: kT[e, i] = sum_d Wk[d, e] * x_key[i, d]
                for et in range(ET):
                    wk_f = stage_pool.tile([P, DT, P], F32, tag="wf",
                                           name=f"wkf{et}")
                    wk_r = stage_pool.tile([P, DT, P], F32R, tag="wr",
                                           name=f"wkr{et}")
                    nc.sync.dma_start(
                        out=wk_f[:],
                        in_=wk_p[:, et * P:(et + 1) * P].rearrange(
                            "(dt p) e -> p dt e", p=P))
                    nc.vector.tensor_copy(wk_r[:], wk_f[:])
                    pss = [psb[ch * 2 + (et % 2)] for ch in range(2)]
                    for d in range(DT):
                        for ch in range(2):
                            nc.tensor.matmul(
                                pss[ch][:],
                                wk_r[:, d, :],
                                xT_keys[:, d, 0, ch * 512:(ch + 1) * 512],
                                start=(d == 0), stop=(d == DT - 1))
                    for ch in range(2):
                        nc.vector.tensor_copy(
                            kT[:, et, ch * 512:(ch + 1) * 512], pss[ch][:])

                # A3: v[i, e] = sum_d x_key[i, d] * Wv[d, e]   (8 psum banks)
                for eb in range(2):
                    for d in range(DT):
                        wv_f = stage_pool.tile([P, 512], F32, tag="wf",
                                               name=f"wvf{eb}_{d}")
                        wv_r = stage_pool.tile([P, 512], F32R, tag="wr",
                                               name=f"wvr{eb}_{d}")
                        nc.sync.dma_start(
                            out=wv_f[:],
                            in_=wv_p[d * P:(d + 1) * P,
                                     eb * 512:(eb + 1) * 512])
                        nc.vector.tensor_copy(wv_r[:], wv_f[:])
                        for st in range(HKT):
                            nc.tensor.matmul(
                                psb[st][:],
                                xT_keys[:, d, 0, st * P:(st + 1) * P],
                                wv_r[:],
                                start=(d == 0), stop=(d == DT - 1))
                    for st in range(HKT):
                        nc.vector.tensor_copy(
                            vv[:, st, eb * 512:(eb + 1) * 512], psb[st][:])

                # A4: qT[e, s] = sum_d Wq[d, e] * x[s, d]  (all queries)
                for et in range(ET):
                    wq_f = stage_pool.tile([P, DT, P], F32, tag="wf",
                                           name=f"wqf{et}")
                    wq_r = stage_pool.tile([P, DT, P], F32R, tag="wr",
                                           name=f"wqr{et}")
                    nc.sync.dma_start(
                        out=wq_f[:],
                        in_=wq_p[:, et * P:(et + 1) * P].rearrange(
                            "(dt p) e -> p dt e", p=P))
                    nc.vector.tensor_copy(wq_r[:], wq_f[:])
                    pss = [psb[ch * 2 + (et % 2)] for ch in range(4)]
                    for d in range(DT):
                        for ch in range(4):
                            nc.tensor.matmul(
                                pss[ch][:],
                                wq_r[:, d, :],
                                xT[:, d, ch * 512:(ch + 1) * 512],
                                start=(d == 0), stop=(d == DT - 1))
                    for ch in range(4):
                        nc.vector.tensor_copy(
                            qT[:, et, ch * 512:(ch + 1) * 512], pss[ch][:])

            # ================= Phase B: causal attention =================
            with (
                tc.tile_pool(name="sc_pool", bufs=2) as sc_pool,
                tc.tile_pool(name="at_pool", bufs=2) as at_pool,
                tc.tile_pool(name="atT_pool", bufs=4) as atT_pool,
                tc.tile_pool(name="st_pool", bufs=4) as st_pool,
                tc.tile_pool(name="ob_pool", bufs=2) as ob_pool,
                tc.tile_pool(name="psS_pool", bufs=2, space="PSUM") as psS_pool,
                tc.tile_pool(name="psA_pool", bufs=2, space="PSUM") as psA_pool,
                tc.tile_pool(name="psO_pool", bufs=1, space="PSUM") as psO_pool,
            ):
                for j in range(NQB):
                    nkb = j // 2 + 1          # valid compacted key blocks
                    ncols = nkb * P
                    nch = _ceil_div(ncols, 512)
                    scores = sc_pool.tile([P, HKT * P], F32, tag="scores",
                                          name=f"scores{j}")
                    attn = at_pool.tile([P, HKT * P], BF16, tag="attn",
                                        name=f"attn{j}")

                    # scores = qT[:, j-block]^T @ kT  (contract over e)
                    for ch in range(nch):
                        ncc = min(512, ncols - ch * 512)
                        psS = psS_pool.tile([P, 512], F32, tag=f"psS{ch % 2}",
                                            name=f"psS{j}_{ch}")
                        for et in range(ET):
                            nc.tensor.matmul(
                                psS[:, :ncc],
                                qT[:, et, j * P:(j + 1) * P],
                                kT[:, et, ch * 512:ch * 512 + ncc],
                                start=(et == 0), stop=(et == ET - 1))
                        # evict to scores; boundary block gets the causal mask
                        lo, hi = ch * 512, ch * 512 + ncc
                        if hi == ncols:
                            if ncc > P:
                                nc.vector.tensor_copy(scores[:, lo:hi - P],
                                                      psS[:, :ncc - P])
                            nc.vector.tensor_add(
                                scores[:, hi - P:hi],
                                psS[:, ncc - P:ncc],
                                mask_sb[:, j % 2, :])
                        else:
                            nc.vector.tensor_copy(scores[:, lo:hi],
                                                  psS[:, :ncc])

                    # softmax over the valid region
                    m_t = st_pool.tile([P, 1], F32, tag="m", name=f"m{j}")
                    neg_t = st_pool.tile([P, 1], F32, tag="neg", name=f"neg{j}")
                    l_t = st_pool.tile([P, 1], F32, tag="l", name=f"l{j}")
                    nc.vector.reduce_max(m_t[:], scores[:, :ncols],
                                         axis=mybir.AxisListType.X)
                    nc.vector.tensor_scalar_mul(neg_t[:], m_t[:], -INV_SQRT_D)
                    nc.scalar.activation(
                        attn[:, :ncols], scores[:, :ncols],
                        mybir.ActivationFunctionType.Exp,
                        bias=neg_t[:], scale=INV_SQRT_D, accum_out=l_t[:])
                    nc.sync.dma_start(out=m_p[j * P:(j + 1) * P, :], in_=m_t[:])
                    nc.sync.dma_start(out=l_p[j * P:(j + 1) * P, :], in_=l_t[:])

                    # o = attn @ v   (transpose attn blocks, contract over keys)
                    atTs = []
                    for kb in range(nkb):
                        psA = psA_pool.tile([P, P], BF16, tag="psA",
                                            name=f"psA{j}_{kb}")
                        atT = atT_pool.tile([P, P], BF16, tag="atT",
                                            name=f"atT{j}_{kb}")
                        nc.tensor.transpose(
                            psA[:], attn[:, kb * P:(kb + 1) * P], ident_bf[:])
                        nc.vector.tensor_copy(atT[:], psA[:])
                        atTs.append(atT)
                    psO = [psO_pool.tile([P, 512], F32, tag=f"psO{eb}",
                                         name=f"psO{j}_{eb}")
                           for eb in range(2)]
                    for kb in range(nkb):
                        for eb in range(2):
                            nc.tensor.matmul(
                                psO[eb][:],
                                atTs[kb][:],
                                vv[:, kb, eb * 512:(eb + 1) * 512],
                                start=(kb == 0), stop=(kb == nkb - 1))
                    for eb in range(2):
                        o_sb = ob_pool.tile([P, 512], F32, tag="o",
                                            name=f"o{j}_{eb}")
                        nc.vector.tensor_copy(o_sb[:], psO[eb][:])
                        nc.sync.dma_start(
                            out=o_p[j * P:(j + 1) * P,
                                    eb * 512:(eb + 1) * 512],
                            in_=o_sb[:])
    nc.finalize()
    return nc


def _boundary_masks(c):
    """mask[row, par, i]: 0 if compacted key i is causally valid for local
    query row `row` of an even (par=0) / odd (par=1) query block, else -1e30.

    For parity-1 cores, x rows arrive pair-swapped, so the query at local
    position `row` is global row 128*j + r_local with
    r_local = row+1 (even row) / row-1 (odd row). Key i is global row
    256*(j//2) + 2*i + c. Valid iff 2*i + c <= par*128 + r_local.
    """
    mask = np.full((P, 2, P), NEG, dtype=np.float32)
    for row in range(P):
        r_local = row if c == 0 else (row + 1 if row % 2 == 0 else row - 1)
        for par in range(2):
            lim = (par * P + r_local - c) // 2
            if lim >= 0:
                mask[row, par, :min(lim + 1, P)] = 0.0
    return mask


_PAIRSWAP = np.arange(S).reshape(-1, 2)[:, ::-1].reshape(-1)


def _make_in_maps(x, Wq, Wk, Wv):
    x = np.asarray(x, dtype=np.float32)
    Wq = np.ascontiguousarray(np.asarray(Wq, dtype=np.float32))
    Wk = np.ascontiguousarray(np.asarray(Wk, dtype=np.float32))
    Wv = np.ascontiguousarray(np.asarray(Wv, dtype=np.float32))
    masks = [_boundary_masks(0), _boundary_masks(1)]
    in_maps = []
    for core in range(8):
        b, c = core // 2, core % 2
        xb = x[b] if c == 0 else x[b][_PAIRSWAP]
        in_maps.append({
            "x": np.ascontiguousarray(xb),
            "wq": Wq, "wk": Wk, "wv": Wv,
            "mask": masks[c],
        })
    return in_maps


def _combine(res):
    out = np.empty((B, S, D), dtype=np.float32)
    for b in range(B):
        r0, r1 = res.results[2 * b], res.results[2 * b + 1]
        o0, m0, l0 = r0["o"], r0["m"], r0["l"]
        # parity-1 core computed on pair-swapped query rows; swap back
        o1 = r1["o"][_PAIRSWAP]
        m1 = r1["m"][_PAIRSWAP]
        l1 = r1["l"][_PAIRSWAP]
        ms0 = m0.astype(np.float64) * INV_SQRT_D
        ms1 = m1.astype(np.float64) * INV_SQRT_D
        mm = np.maximum(ms0, ms1)
        w0 = np.exp(ms0 - mm)
        w1 = np.exp(ms1 - mm)
        num = w0 * o0.astype(np.float64) + w1 * o1.astype(np.float64)
        den = w0 * l0.astype(np.float64) + w1 * l1.astype(np.float64)
        out[b] = (num / den).astype(np.float32)
    return out


def kernel(x, Wq, Wk, Wv):
    global _CACHED_NC
    if _CACHED_NC is None:
        _CACHED_NC = build_nc()
    in_maps = _make_in_maps(x, Wq, Wk, Wv)
    res = run_bass_kernel_spmd(_CACHED_NC, in_maps, list(range(8)))
    return _combine(res)
